# revision 31
# baseline (speedup 1.0000x reference)
"""Causal single-head attention (B=8, T=2048, C=512, D=64) on 8 trn2 NeuronCores.

Sharding: data-parallel over batch — core b computes the full causal attention
for x[b]; the small projection weights are replicated to every core. No
collectives are needed, and the final output is gathered on the host by
stacking the 8 per-core results.

All on-chip storage is bf16 (PSUM accumulation stays fp32): bf16 matmuls
stream 1 column/cycle on the PE regardless of operand width or contraction
depth (unlike fp32r, which needs strided APs, K=128 and >=256-wide moving
operands for full rate), input DMA bytes halve, and LDWEIGHTS time halves.
Verified numerics: worst-case rel err ~7e-3 vs the fp32 reference.

Host-side prep (layout only, no FLOPs): x[b] is passed pre-transposed as
xT [C, T]; the first 512 t-columns are interleaved with the fused Wq|Wk
weights into one contiguous "xin" blob so a single DMA delivers everything
the first projection group needs; the remaining t-columns ship t-group-major
so group g+1's data lands before group g+2's. The causal triangle rides as a
[128,128] blob; ones/identity constants are memset on-chip.

Per-core dataflow:
  1. qk [128, T] = wqkv.T @ xT per 512-wide slice (rows 0:64 = Q^T,
     64:128 = K^T; the K^T half is re-based to partition 0 by a SBUF->SBUF
     DMA since matmul operands must share a base partition);
     V [t, c] tiles = xT_chunk.T @ Wv_chunk.
  2. per query slice s (512 wide), per key chunk j (128):
       ST [tk=128, tq] = matmul(lhsT=kt[:, j], rhs=qk[0:64, s])   (K=64)
     restricted to the causally valid columns (band chunks start at 128*r);
       E  = exp(0.125 * ST)  (ACT, PSUM->SBUF bf16); the [128,128] diagonal
                             block is multiplied by the triangle mask on DVE.
                             No max-subtraction: scores ~ N(0,1).
       esum += E             elementwise on DVE (bf16); ONE ones.T @ esum
                             matmul per slice then yields the row sums Z,
                             keeping the per-round PE stream at 5 matmuls.
       out[tq=128, c=512] += matmul(lhsT=E[:, k*128:...], rhs=V_j)
     Outputs are copied out of PSUM unnormalized (ACT/DVE alternating, cast
     to bf16) as soon as each 128-row block's accumulation finishes, then
     scaled by 1/Z (transposed to a per-partition column via tiny PE
     transposes, reciprocal on DVE) and DMA'd per 128-row block as bf16;
     the host upcasts to fp32.

Performance notes (measured on trn2 via NTFF profiles):
  - ST matmuls are software-pipelined DEPTH=3 ahead so the PE never waits
    on the ACT exp; output DMAs alternate queues.
  - The PE p-state ramps (0.65 -> 1.2 -> 2.4 GHz over ~3us of continuous
    work), so keeping the PE gap-free matters more than instruction count.
  - V-projection PSUM tiles come from the (otherwise idle during the
    projection phase) o-pool so accumulation-group starts never wait on the
    PSUM->SBUF cast of two groups earlier.
"""

import os
import sys

if "/opt/trn_rl_repo" not in sys.path:
    sys.path.insert(0, "/opt/trn_rl_repo")

import numpy as np

import concourse.tile as tile
from concourse import bacc, mybir
from concourse.bass import ts

B, T, C_IN, C_OUT, D = 8, 2048, 512, 512, 64
NT = T // 128  # 16 key chunks / t tiles
NC = C_IN // 128  # 4 c_in chunks
NS = T // 512  # 4 query slices
F32 = mybir.dt.float32
BF = mybir.dt.bfloat16

last_result = None  # BassKernelResults of the most recent run (for test harness)


def _emit(tc):
    nc = tc.nc

    # xin: per c-chunk j, [wqkv_j | xT_j[:, 0:512]] interleaved, 640 cols each
    xin = nc.dram_tensor("xin", [C_IN, 640], BF, kind="ExternalInput").ap()
    # xt_rest: t-group-major [3, C, 512] so group g data lands in g order
    xt_rest = nc.dram_tensor("xt_rest", [3 * C_IN, 512], BF, kind="ExternalInput").ap()
    wv = nc.dram_tensor("wv", [C_IN, C_OUT], BF, kind="ExternalInput").ap()
    tri = nc.dram_tensor("tri", [128, 128], BF, kind="ExternalInput").ap()
    out = nc.dram_tensor("out", [T, C_OUT], BF, kind="ExternalOutput").ap()

    with (
        tc.tile_pool(name="persist", bufs=1) as pp,
        tc.tile_pool(name="epool", bufs=4) as ep,
        tc.tile_pool(name="espool", bufs=2) as esp,
        tc.tile_pool(name="opool", bufs=2) as outp,
        tc.tile_pool(name="rzp", bufs=2) as rzp,
        tc.tile_pool(name="stp", bufs=4, space="PSUM") as stp,
        tc.tile_pool(name="op", bufs=4, space="PSUM") as op,
    ):
        # ---- persistent SBUF tensors ----
        xin_sb = pp.tile([128, NC * 640], BF, tag="xin")  # c-chunk j at ts(j, 640)
        xt_sb = pp.tile([128, NC * 1536], BF, tag="xt")  # c-chunk j at ts(j, 1536)
        qk_sb = pp.tile([128, T], BF, tag="qk")  # rows 0:64 Q^T, 64:128 K^T
        kt_sb = pp.tile([64, T], BF, tag="kt")  # K^T re-based to partition 0
        v_sb = pp.tile([128, NT * C_OUT], BF, tag="v")  # tk-tile j at ts(j, 512)
        wv_sb = pp.tile([128, NC * C_OUT], BF, tag="wv")
        tri_sb = pp.tile([128, 128], BF, tag="tri")
        ones_sb = pp.tile([128, 1], BF, tag="ones")
        id_sb = pp.tile([1, 1], F32, tag="id")

        nc.gpsimd.memset(ones_sb[:], 1.0)
        nc.gpsimd.memset(id_sb[:], 1.0)

        def wqkv_ap(j):
            return xin_sb[:, 640 * j : 640 * j + 128]

        def xcol(j, t0, w):
            """xT chunk j columns [t0, t0+w) — never straddles the 512 line."""
            if t0 < 512:
                c0 = 640 * j + 128 + t0
                return xin_sb[:, c0 : c0 + w]
            c0 = 1536 * j + (t0 - 512)
            return xt_sb[:, c0 : c0 + w]

        # ---- input DMAs: few large contiguous blocks, parallel issue ----
        # per-chunk xin DMAs spread across all three issue queues: the first
        # projection matmul only needs chunk 0, and the chunks land ~1 issue
        # apart instead of serializing behind one queue
        xin_q = [nc.sync, nc.scalar, nc.gpsimd, nc.sync]
        for j in range(NC):
            xin_q[j].dma_start(
                xin_sb[:, ts(j, 640)], xin[128 * j : 128 * (j + 1), :]
            )
        # wv ships per chunk so V-projection matmul j only waits on chunk j
        for j in range(NC):
            eng = nc.scalar if j % 2 == 0 else nc.gpsimd
            eng.dma_start(
                wv_sb[:, ts(j, 512)], wv[128 * j : 128 * (j + 1), :]
            )
        nc.gpsimd.dma_start(tri_sb[:], tri)
        xt_sb4 = xt_sb.rearrange("p (j r d) -> p j r d", r=3, d=512)
        for g in range(3):
            nc.sync.dma_start(
                xt_sb4[:, :, g, :],
                xt_rest[C_IN * g : C_IN * (g + 1), :].rearrange(
                    "(j p) d -> p j d", p=128
                ),
            )

        # ---- projections, per t-group g ----
        for g in range(4):
            qk_ps = stp.tile([128, 512], F32, tag="st", name="qk_ps")
            for j in range(NC):
                nc.tensor.matmul(
                    qk_ps[:],
                    wqkv_ap(j),
                    xcol(j, 512 * g, 512),
                    start=(j == 0),
                    stop=(j == NC - 1),
                )
            nc.vector.tensor_copy(qk_sb[:, ts(g, 512)], qk_ps[:])
            # matmul operands must share a base partition: move the K^T half
            # down to partitions 0:64 with a SBUF->SBUF DMA (engines can't
            # shift partitions; the DMA hides under later projection groups)
            nc.gpsimd.dma_start(kt_sb[:, ts(g, 512)], qk_sb[64:128, ts(g, 512)])
            for i in range(4 * g, 4 * g + 4):
                v_ps = op.tile([128, 512], F32, tag="o", name="v_ps")
                for j in range(NC):
                    nc.tensor.matmul(
                        v_ps[:],
                        xcol(j, 128 * i, 128),
                        wv_sb[:, ts(j, 512)],
                        start=(j == 0),
                        stop=(j == NC - 1),
                    )
                # last group's copies stay off ACT so the first exps of the
                # attention phase are not queued behind them
                if i % 2 == 0 or g == 3:
                    nc.vector.tensor_copy(v_sb[:, ts(i, 512)], v_ps[:])
                else:
                    nc.scalar.copy(v_sb[:, ts(i, 512)], v_ps[:])

        # ---- attention ----
        def emit_st(s, j):
            r = j - 4 * s  # band index; valid query cols start at 128*r
            lo = 128 * r if r >= 0 else 0
            st_ps = stp.tile([128, 512], F32, tag="st", name="st_ps")
            nc.tensor.matmul(
                st_ps[:, lo:512],
                kt_sb[:, ts(j, 128)],
                qk_sb[0:64, 512 * s + lo : 512 * (s + 1)],
                start=True,
                stop=True,
            )
            return st_ps

        DEPTH = 4
        pend = {0: {j: emit_st(0, j) for j in range(DEPTH)}}
        o_bigs = {}
        n_out = 0
        for s in range(NS):
            nj = 4 * s + 4
            st_tiles = pend.pop(s)
            o_ps = [
                op.tile([128, 512], F32, tag="o", name=f"o_ps{k}") for k in range(4)
            ]
            esum = esp.tile([128, 512], BF, name="esum")
            late_es = {}
            o_bigs[s] = outp.tile([128, 2048], BF, name="o_big")
            for j in range(nj):
                jn = j + DEPTH
                if jn < nj:
                    st_tiles[jn] = emit_st(s, jn)
                elif s + 1 < NS and jn - nj < DEPTH:
                    pend.setdefault(s + 1, {})[jn - nj] = emit_st(s + 1, jn - nj)
                st_ps = st_tiles.pop(j)
                e = ep.tile([128, 512], BF, name="e")
                r = j - 4 * s
                lo = 128 * r if r >= 0 else 0
                nc.scalar.activation(
                    e[:, lo:512],
                    st_ps[:, lo:512],
                    mybir.ActivationFunctionType.Exp,
                    scale=0.125,
                )
                if r >= 0:
                    nc.vector.tensor_mul(e[:, ts(r, 128)], e[:, ts(r, 128)], tri_sb[:])
                # Z accumulation runs on DVE (elementwise tile sum) so the PE
                # round stays at 5 matmuls; one matmul per slice reduces esum.
                # The last slice's final two band chunks skip the serial DVE
                # add chain and feed Z directly (shortens the kernel tail)
                if s == NS - 1 and r >= 2:
                    late_es[j] = e
                elif j == 0:
                    nc.vector.tensor_copy(esum[:], e[:])
                else:
                    nc.vector.tensor_add(
                        esum[:, lo:512], esum[:, lo:512], e[:, lo:512]
                    )
                o_big = o_bigs[s]
                for k in range(4):
                    m = 4 * s + k
                    if j <= m:
                        nc.tensor.matmul(
                            o_ps[k][:],
                            e[:, ts(k, 128)],
                            v_sb[:, ts(j, 512)],
                            start=(j == 0),
                            stop=(j == m),
                        )
                        if j == m and s != NS - 1:
                            # accumulation done: copy out unnormalized now so
                            # the PSUM bank frees before the 1/Z chain
                            # finishes. The last slice skips this and fuses
                            # normalize+copy instead (no successor needs the
                            # PSUM banks, and it halves the tail chain)
                            nc.vector.tensor_copy(
                                o_big[:, ts(k, 512)], o_ps[k][:]
                            )
            # Z: one PE reduction of esum, row to SBUF (DVE), transpose to
            # columns (PE), reciprocal (DVE). z_ps lives only briefly, so it
            # shares the stp pool (frees a PSUM bank for the DEPTH=4 pipeline)
            z_ps = stp.tile([1, 512], F32, tag="st", name="z_ps")
            zmms = [(esum[:], 0)] + sorted(
                ((e_[:, 128 * (jj - 4 * s) : 512], 128 * (jj - 4 * s))
                 for jj, e_ in late_es.items()),
                key=lambda t: t[1],
            )
            for i_, (ap, lo_) in enumerate(zmms):
                nc.tensor.matmul(
                    z_ps[0:1, lo_:512],
                    ones_sb[:, 0:1],
                    ap,
                    start=(i_ == 0),
                    stop=(i_ == len(zmms) - 1),
                )
            z_sb = rzp.tile([1, 512], F32, tag="z", name="z_sb")
            rzt_ps = stp.tile([128, 4], F32, tag="st", name="rzt_ps")
            rzt = rzp.tile([128, 4], F32, tag="rzt", name="rzt")
            nc.vector.tensor_copy(z_sb[:], z_ps[0:1, :])
            for k in range(4):
                nc.tensor.transpose(
                    rzt_ps[:, k : k + 1], z_sb[0:1, ts(k, 128)], id_sb[:]
                )
            nc.vector.reciprocal(rzt[:], rzt_ps[:])
            for k in range(4):
                # normalize muls stay off ACT mid-kernel (exp-only queue);
                # the last slice fuses normalize+PSUM-copy two engines wide
                if s == NS - 1:
                    if k % 2 == 1:
                        nc.scalar.activation(
                            o_big[:, ts(k, 512)],
                            o_ps[k][:],
                            mybir.ActivationFunctionType.Copy,
                            scale=rzt[:, k : k + 1],
                        )
                    else:
                        nc.vector.tensor_scalar_mul(
                            o_big[:, ts(k, 512)], o_ps[k][:], rzt[:, k : k + 1]
                        )
                else:
                    nc.vector.tensor_scalar_mul(
                        o_big[:, ts(k, 512)], o_big[:, ts(k, 512)], rzt[:, k : k + 1]
                    )
                r0 = 512 * s + 128 * k
                # mid-kernel output DMAs issue from sync only (scalar issues
                # would block the exps); the last slice can use both queues
                if s == NS - 1:
                    eng = nc.sync if n_out % 2 == 0 else nc.scalar
                else:
                    eng = nc.sync
                eng.dma_start(out[r0 : r0 + 128, :], o_big[:, ts(k, 512)])
                n_out += 1


def build_nc():
    nc = bacc.Bacc(
        "TRN2",
        target_bir_lowering=False,
        debug=False,
        enable_asserts=False,
        num_devices=B,
    )
    with tile.TileContext(nc) as tc:
        _emit(tc)
    nc.compile()
    return nc


_nc_cache = {}


def _install_ntff_hook():
    """Provide antenv.axon_hooks (absent in this image) so that
    run_bass_kernel_spmd(trace=True) can capture NTFF profiles via the
    axon ctypes hook from trn_agent_boot."""
    import types

    if "antenv.axon_hooks" in sys.modules:
        return
    mod = types.ModuleType("antenv.axon_hooks")
    holder = [None]
    mod.set_axon_ntff_profile_hook = lambda h: holder.__setitem__(0, h)
    mod.get_axon_ntff_profile_hook = lambda: holder[0]
    sys.modules["antenv.axon_hooks"] = mod
    try:
        from trn_agent_boot.trn_boot import _ntff_profile_via_ctypes

        holder[0] = _ntff_profile_via_ctypes("/opt/axon/libaxon_pjrt.so")
    except Exception as e:  # degrade to no tracing
        print(f"ntff hook install failed: {e}", file=sys.stderr)


def kernel(x, Wq, Wk, Wv):
    import ml_dtypes

    from concourse import bass_utils

    bf = ml_dtypes.bfloat16
    x = np.asarray(x, dtype=np.float32)
    Wq = np.asarray(Wq, dtype=np.float32)
    Wk = np.asarray(Wk, dtype=np.float32)
    Wv = np.asarray(Wv, dtype=np.float32)
    assert x.shape == (B, T, C_IN), x.shape

    if "nc" not in _nc_cache:
        _nc_cache["nc"] = build_nc()
    nc = _nc_cache["nc"]

    xt = np.ascontiguousarray(x.transpose(0, 2, 1)).astype(bf)  # [B, C, T]
    wqkv = np.concatenate([Wq, Wk], axis=1).astype(bf)  # [C, 128]
    wv_bf = np.ascontiguousarray(Wv).astype(bf)
    p = np.arange(128)[:, None]
    f = np.arange(128)[None, :]
    tri = (p <= f).astype(bf)  # key p valid for query f when p <= f
    in_maps = []
    for b in range(B):
        xin = np.empty((C_IN, 640), dtype=bf)
        xin[:, 0:128] = wqkv
        xin[:, 128:640] = xt[b, :, 0:512]
        # [3, C, 512] t-group-major remainder
        xr = np.ascontiguousarray(
            xt[b, :, 512:2048].reshape(C_IN, 3, 512).transpose(1, 0, 2)
        ).reshape(3 * C_IN, 512)
        in_maps.append(
            {"xin": xin, "xt_rest": xr, "wv": wv_bf, "tri": tri}
        )
    trace = os.environ.get("KERNEL_TRACE", "0") == "1"
    if trace:
        _install_ntff_hook()
    res = bass_utils.run_bass_kernel_spmd(
        nc, in_maps, core_ids=list(range(B)), trace=trace
    )
    global last_result
    last_result = res
    return np.stack([r["out"] for r in res.results], axis=0).astype(np.float32)


# revision 33
# speedup vs baseline: 1.0414x; 1.0414x over previous
"""Causal single-head attention (B=8, T=2048, C=512, D=64) on 8 trn2 NeuronCores.

Sharding: data-parallel over batch — core b computes the full causal attention
for x[b]; the small projection weights are replicated to every core. No
collectives are needed, and the final output is gathered on the host by
stacking the 8 per-core results.

All on-chip storage is bf16 (PSUM accumulation stays fp32): bf16 matmuls
stream 1 column/cycle on the PE regardless of operand width or contraction
depth (unlike fp32r, which needs strided APs, K=128 and >=256-wide moving
operands for full rate), input DMA bytes halve, and LDWEIGHTS time halves.
Verified numerics: worst-case rel err ~7e-3 vs the fp32 reference.

Host-side prep (layout only, no FLOPs): x[b] is passed pre-transposed as
xT [C, T]; the first 512 t-columns are interleaved with the fused Wq|Wk
weights into one contiguous "xin" blob so a single DMA delivers everything
the first projection group needs; the remaining t-columns ship t-group-major
so group g+1's data lands before group g+2's. The causal triangle rides as a
[128,128] blob; ones/identity constants are memset on-chip.

Per-core dataflow:
  1. qk [128, T] = wqkv.T @ xT per 512-wide slice (rows 0:64 = Q^T,
     64:128 = K^T; the K^T half is re-based to partition 0 by a SBUF->SBUF
     DMA since matmul operands must share a base partition);
     V [t, c] tiles = xT_chunk.T @ Wv_chunk.
  2. per query slice s (512 wide), per key chunk j (128):
       ST [tk=128, tq] = matmul(lhsT=kt[:, j], rhs=qk[0:64, s])   (K=64)
     restricted to the causally valid columns (band chunks start at 128*r);
       E  = exp(0.125 * ST)  (ACT, PSUM->SBUF bf16); the [128,128] diagonal
                             block is multiplied by the triangle mask on DVE.
                             No max-subtraction: scores ~ N(0,1).
       esum += E             elementwise on DVE (bf16); ONE ones.T @ esum
                             matmul per slice then yields the row sums Z,
                             keeping the per-round PE stream at 5 matmuls.
       out[tq=128, c=512] += matmul(lhsT=E[:, k*128:...], rhs=V_j)
     Outputs are copied out of PSUM unnormalized (ACT/DVE alternating, cast
     to bf16) as soon as each 128-row block's accumulation finishes, then
     scaled by 1/Z (transposed to a per-partition column via tiny PE
     transposes, reciprocal on DVE) and DMA'd per 128-row block as bf16;
     the host upcasts to fp32.

Performance notes (measured on trn2 via NTFF profiles):
  - ST matmuls are software-pipelined DEPTH=3 ahead so the PE never waits
    on the ACT exp; output DMAs alternate queues.
  - The PE p-state ramps (0.65 -> 1.2 -> 2.4 GHz over ~3us of continuous
    work), so keeping the PE gap-free matters more than instruction count.
  - V-projection PSUM tiles come from the (otherwise idle during the
    projection phase) o-pool so accumulation-group starts never wait on the
    PSUM->SBUF cast of two groups earlier.
"""

import os
import sys

if "/opt/trn_rl_repo" not in sys.path:
    sys.path.insert(0, "/opt/trn_rl_repo")

import numpy as np

import concourse.tile as tile
from concourse import bacc, mybir
from concourse.bass import ts

B, T, C_IN, C_OUT, D = 8, 2048, 512, 512, 64
NT = T // 128  # 16 key chunks / t tiles
NC = C_IN // 128  # 4 c_in chunks
NS = T // 512  # 4 query slices
F32 = mybir.dt.float32
BF = mybir.dt.bfloat16

last_result = None  # BassKernelResults of the most recent run (for test harness)


def _emit(tc):
    nc = tc.nc

    # xin: per c-chunk j, [wqkv_j | xT_j[:, 0:512]] interleaved, 640 cols each
    xin = nc.dram_tensor("xin", [C_IN, 640], BF, kind="ExternalInput").ap()
    # xt_rest: t-group-major [3, C, 512] so group g data lands in g order
    xt_rest = nc.dram_tensor("xt_rest", [3 * C_IN, 512], BF, kind="ExternalInput").ap()
    wv = nc.dram_tensor("wv", [C_IN, C_OUT], BF, kind="ExternalInput").ap()
    tri = nc.dram_tensor("tri", [128, 128], BF, kind="ExternalInput").ap()
    out = nc.dram_tensor("out", [T, C_OUT], BF, kind="ExternalOutput").ap()

    with (
        tc.tile_pool(name="persist", bufs=1) as pp,
        tc.tile_pool(name="epool", bufs=4) as ep,
        tc.tile_pool(name="espool", bufs=2) as esp,
        tc.tile_pool(name="opool", bufs=2) as outp,
        tc.tile_pool(name="rzp", bufs=2) as rzp,
        tc.tile_pool(name="stp", bufs=4, space="PSUM") as stp,
        tc.tile_pool(name="op", bufs=4, space="PSUM") as op,
    ):
        # ---- persistent SBUF tensors ----
        xin_sb = pp.tile([128, NC * 640], BF, tag="xin")  # c-chunk j at ts(j, 640)
        xt_sb = pp.tile([128, NC * 1536], BF, tag="xt")  # c-chunk j at ts(j, 1536)
        qk_sb = pp.tile([128, T], BF, tag="qk")  # rows 0:64 Q^T, 64:128 K^T
        kt_sb = pp.tile([64, T], BF, tag="kt")  # K^T re-based to partition 0
        v_sb = pp.tile([128, NT * C_OUT], BF, tag="v")  # tk-tile j at ts(j, 512)
        wv_sb = pp.tile([128, NC * C_OUT], BF, tag="wv")
        tri_sb = pp.tile([128, 128], BF, tag="tri")
        ones_sb = pp.tile([128, 1], BF, tag="ones")
        id_sb = pp.tile([1, 1], F32, tag="id")

        nc.gpsimd.memset(ones_sb[:], 1.0)
        nc.gpsimd.memset(id_sb[:], 1.0)

        def wqkv_ap(j):
            return xin_sb[:, 640 * j : 640 * j + 128]

        def xcol(j, t0, w):
            """xT chunk j columns [t0, t0+w) — never straddles the 512 line."""
            if t0 < 512:
                c0 = 640 * j + 128 + t0
                return xin_sb[:, c0 : c0 + w]
            c0 = 1536 * j + (t0 - 512)
            return xt_sb[:, c0 : c0 + w]

        # ---- input DMAs: few large contiguous blocks, parallel issue ----
        # per-chunk xin DMAs on sync (arrivals pace the first matmul group);
        # wv ships whole on scalar in parallel and lands before the first
        # V-projection needs it; tri rides the gpsimd queue
        for j in range(NC):
            nc.sync.dma_start(
                xin_sb[:, ts(j, 640)], xin[128 * j : 128 * (j + 1), :]
            )
        nc.scalar.dma_start(
            wv_sb.rearrange("p (j d) -> p j d", d=512),
            wv.rearrange("(j p) d -> p j d", p=128),
        )
        nc.gpsimd.dma_start(tri_sb[:], tri)
        xt_sb4 = xt_sb.rearrange("p (j r d) -> p j r d", r=3, d=512)
        for g in range(3):
            nc.sync.dma_start(
                xt_sb4[:, :, g, :],
                xt_rest[C_IN * g : C_IN * (g + 1), :].rearrange(
                    "(j p) d -> p j d", p=128
                ),
            )

        # ---- projections, per t-group g ----
        for g in range(4):
            qk_ps = stp.tile([128, 512], F32, tag="st", name="qk_ps")
            for j in range(NC):
                nc.tensor.matmul(
                    qk_ps[:],
                    wqkv_ap(j),
                    xcol(j, 512 * g, 512),
                    start=(j == 0),
                    stop=(j == NC - 1),
                )
            nc.vector.tensor_copy(qk_sb[:, ts(g, 512)], qk_ps[:])
            # matmul operands must share a base partition: move the K^T half
            # down to partitions 0:64 with a SBUF->SBUF DMA (engines can't
            # shift partitions; the DMA hides under later projection groups)
            nc.gpsimd.dma_start(kt_sb[:, ts(g, 512)], qk_sb[64:128, ts(g, 512)])
            for i in range(4 * g, 4 * g + 4):
                v_ps = op.tile([128, 512], F32, tag="o", name="v_ps")
                for j in range(NC):
                    nc.tensor.matmul(
                        v_ps[:],
                        xcol(j, 128 * i, 128),
                        wv_sb[:, ts(j, 512)],
                        start=(j == 0),
                        stop=(j == NC - 1),
                    )
                # last group's copies stay off ACT so the first exps of the
                # attention phase are not queued behind them
                if i % 2 == 0 or g == 3:
                    nc.vector.tensor_copy(v_sb[:, ts(i, 512)], v_ps[:])
                else:
                    nc.scalar.copy(v_sb[:, ts(i, 512)], v_ps[:])

        # ---- attention ----
        def emit_st(s, j):
            r = j - 4 * s  # band index; valid query cols start at 128*r
            lo = 128 * r if r >= 0 else 0
            st_ps = stp.tile([128, 512], F32, tag="st", name="st_ps")
            nc.tensor.matmul(
                st_ps[:, lo:512],
                kt_sb[:, ts(j, 128)],
                qk_sb[0:64, 512 * s + lo : 512 * (s + 1)],
                start=True,
                stop=True,
            )
            return st_ps

        DEPTH = 4
        pend = {0: {j: emit_st(0, j) for j in range(DEPTH)}}
        o_bigs = {}
        n_out = 0
        for s in range(NS):
            nj = 4 * s + 4
            st_tiles = pend.pop(s)
            o_ps = [
                op.tile([128, 512], F32, tag="o", name=f"o_ps{k}") for k in range(4)
            ]
            esum = esp.tile([128, 512], BF, name="esum")
            late_es = {}
            o_bigs[s] = outp.tile([128, 2048], BF, name="o_big")
            for j in range(nj):
                jn = j + DEPTH
                if jn < nj:
                    st_tiles[jn] = emit_st(s, jn)
                elif s + 1 < NS and jn - nj < DEPTH:
                    pend.setdefault(s + 1, {})[jn - nj] = emit_st(s + 1, jn - nj)
                st_ps = st_tiles.pop(j)
                e = ep.tile([128, 512], BF, name="e")
                r = j - 4 * s
                lo = 128 * r if r >= 0 else 0
                nc.scalar.activation(
                    e[:, lo:512],
                    st_ps[:, lo:512],
                    mybir.ActivationFunctionType.Exp,
                    scale=0.125,
                )
                if r >= 0:
                    nc.vector.tensor_mul(e[:, ts(r, 128)], e[:, ts(r, 128)], tri_sb[:])
                # Z accumulation runs on DVE (elementwise tile sum) so the PE
                # round stays at 5 matmuls; one matmul per slice reduces esum.
                # The last slice's final two band chunks skip the serial DVE
                # add chain and feed Z directly (shortens the kernel tail)
                if s == NS - 1 and r >= 2:
                    late_es[j] = e
                elif j == 0:
                    nc.vector.tensor_copy(esum[:], e[:])
                else:
                    nc.vector.tensor_add(
                        esum[:, lo:512], esum[:, lo:512], e[:, lo:512]
                    )
                o_big = o_bigs[s]
                for k in range(4):
                    m = 4 * s + k
                    if j <= m:
                        nc.tensor.matmul(
                            o_ps[k][:],
                            e[:, ts(k, 128)],
                            v_sb[:, ts(j, 512)],
                            start=(j == 0),
                            stop=(j == m),
                        )
                        if j == m and s != NS - 1:
                            # accumulation done: copy out unnormalized now so
                            # the PSUM bank frees before the 1/Z chain
                            # finishes. The last slice skips this and fuses
                            # normalize+copy instead (no successor needs the
                            # PSUM banks, and it halves the tail chain)
                            nc.vector.tensor_copy(
                                o_big[:, ts(k, 512)], o_ps[k][:]
                            )
            # Z: one PE reduction of esum, row to SBUF (DVE), transpose to
            # columns (PE), reciprocal (DVE). z_ps lives only briefly, so it
            # shares the stp pool (frees a PSUM bank for the DEPTH=4 pipeline)
            z_ps = stp.tile([1, 512], F32, tag="st", name="z_ps")
            zmms = [(esum[:], 0)] + sorted(
                ((e_[:, 128 * (jj - 4 * s) : 512], 128 * (jj - 4 * s))
                 for jj, e_ in late_es.items()),
                key=lambda t: t[1],
            )
            for i_, (ap, lo_) in enumerate(zmms):
                nc.tensor.matmul(
                    z_ps[0:1, lo_:512],
                    ones_sb[:, 0:1],
                    ap,
                    start=(i_ == 0),
                    stop=(i_ == len(zmms) - 1),
                )
            z_sb = rzp.tile([1, 512], F32, tag="z", name="z_sb")
            rzt_ps = stp.tile([128, 4], F32, tag="st", name="rzt_ps")
            rzt = rzp.tile([128, 4], F32, tag="rzt", name="rzt")
            # last slice: ACT is idle (exps done, copies fused) — use it for
            # the z copy so the tail chain doesn't queue behind DVE adds
            if s == NS - 1:
                nc.scalar.copy(z_sb[:], z_ps[0:1, :])
            else:
                nc.vector.tensor_copy(z_sb[:], z_ps[0:1, :])
            for k in range(4):
                nc.tensor.transpose(
                    rzt_ps[:, k : k + 1], z_sb[0:1, ts(k, 128)], id_sb[:]
                )
            nc.vector.reciprocal(rzt[:], rzt_ps[:])
            for k in range(4):
                # normalize muls stay off ACT mid-kernel (exp-only queue);
                # the last slice fuses normalize+PSUM-copy two engines wide
                if s == NS - 1:
                    if k % 2 == 1:
                        nc.scalar.activation(
                            o_big[:, ts(k, 512)],
                            o_ps[k][:],
                            mybir.ActivationFunctionType.Copy,
                            scale=rzt[:, k : k + 1],
                        )
                    else:
                        nc.vector.tensor_scalar_mul(
                            o_big[:, ts(k, 512)], o_ps[k][:], rzt[:, k : k + 1]
                        )
                else:
                    nc.vector.tensor_scalar_mul(
                        o_big[:, ts(k, 512)], o_big[:, ts(k, 512)], rzt[:, k : k + 1]
                    )
                r0 = 512 * s + 128 * k
                # mid-kernel output DMAs issue from sync only (scalar issues
                # would block the exps); the last slice can use both queues
                if s == NS - 1:
                    eng = nc.sync if n_out % 2 == 0 else nc.scalar
                else:
                    eng = nc.sync
                eng.dma_start(out[r0 : r0 + 128, :], o_big[:, ts(k, 512)])
                n_out += 1


def build_nc():
    nc = bacc.Bacc(
        "TRN2",
        target_bir_lowering=False,
        debug=False,
        enable_asserts=False,
        num_devices=B,
    )
    with tile.TileContext(nc) as tc:
        _emit(tc)
    nc.compile()
    return nc


_nc_cache = {}


def _install_ntff_hook():
    """Provide antenv.axon_hooks (absent in this image) so that
    run_bass_kernel_spmd(trace=True) can capture NTFF profiles via the
    axon ctypes hook from trn_agent_boot."""
    import types

    if "antenv.axon_hooks" in sys.modules:
        return
    mod = types.ModuleType("antenv.axon_hooks")
    holder = [None]
    mod.set_axon_ntff_profile_hook = lambda h: holder.__setitem__(0, h)
    mod.get_axon_ntff_profile_hook = lambda: holder[0]
    sys.modules["antenv.axon_hooks"] = mod
    try:
        from trn_agent_boot.trn_boot import _ntff_profile_via_ctypes

        holder[0] = _ntff_profile_via_ctypes("/opt/axon/libaxon_pjrt.so")
    except Exception as e:  # degrade to no tracing
        print(f"ntff hook install failed: {e}", file=sys.stderr)


def kernel(x, Wq, Wk, Wv):
    import ml_dtypes

    from concourse import bass_utils

    bf = ml_dtypes.bfloat16
    x = np.asarray(x, dtype=np.float32)
    Wq = np.asarray(Wq, dtype=np.float32)
    Wk = np.asarray(Wk, dtype=np.float32)
    Wv = np.asarray(Wv, dtype=np.float32)
    assert x.shape == (B, T, C_IN), x.shape

    if "nc" not in _nc_cache:
        _nc_cache["nc"] = build_nc()
    nc = _nc_cache["nc"]

    xt = np.ascontiguousarray(x.transpose(0, 2, 1)).astype(bf)  # [B, C, T]
    wqkv = np.concatenate([Wq, Wk], axis=1).astype(bf)  # [C, 128]
    wv_bf = np.ascontiguousarray(Wv).astype(bf)
    p = np.arange(128)[:, None]
    f = np.arange(128)[None, :]
    tri = (p <= f).astype(bf)  # key p valid for query f when p <= f
    in_maps = []
    for b in range(B):
        xin = np.empty((C_IN, 640), dtype=bf)
        xin[:, 0:128] = wqkv
        xin[:, 128:640] = xt[b, :, 0:512]
        # [3, C, 512] t-group-major remainder
        xr = np.ascontiguousarray(
            xt[b, :, 512:2048].reshape(C_IN, 3, 512).transpose(1, 0, 2)
        ).reshape(3 * C_IN, 512)
        in_maps.append(
            {"xin": xin, "xt_rest": xr, "wv": wv_bf, "tri": tri}
        )
    trace = os.environ.get("KERNEL_TRACE", "0") == "1"
    if trace:
        _install_ntff_hook()
    res = bass_utils.run_bass_kernel_spmd(
        nc, in_maps, core_ids=list(range(B)), trace=trace
    )
    global last_result
    last_result = res
    return np.stack([r["out"] for r in res.results], axis=0).astype(np.float32)
